# revision 3
# baseline (speedup 1.0000x reference)
"""DANet3D dual-attention kernel for Trainium2 (8 NeuronCores, Bass/Tile).

Sharding: x -> proj p [2, 64, 8000]; 8 cores = 2 batches x 4 query-blocks
of 2000 positions.  Each core receives the full batch projection (keys /
values / channel attention) plus its own query block and computes its
[64, 2000] slice of the output.

Position attention (per batch), with M = Wq^T Wk, w = Wk^T bq:
  softmax_m( p_n^T M p_m + w.p_m )  ->  flash loop in E^T layout
  F = exp(kp_m . p_n + w.p_m),  kp = M p
  U[65, q] += vt[m, 0:65]^T F[m, q],  vt = [gamma_p*(Wv p + bv) | ones]
  (gamma_p is folded into Wv/bv on the host; the ones column comes from
  the x-buffer's ones row, which is zeroed for padded keys so padding
  self-cancels in both numerator and denominator).

Engine budget (per core, warm): PE ~190k cycles of F/U matmuls is the
long pole; the exp of the 8064x2000 score matrix (16.1M elements) can
only run on ACT (1.2G col/s) and DVE (0.96G col/s) because GPSIMD has no
PSUM port.  The kernel therefore:
  * pipelines F (PSUM slots f0..f3, 2 per sub-iter, reuse distance 2)
    ahead of exp, with U two sub-iters behind, so the PE instruction
    stream has no dependency stalls (HAM stays at K=8/8, 2.4 GHz);
  * splits the 8 exp tiles per key-pair 5:3 between ACT (native Exp with
    the per-key bias in the ACT bias slot) and DVE (Schraudolph exp:
    int16(x*184.665 + (w.p*184.665+16256)) bit-cast to bf16);
  * moves everything else off the critical engines: p^T tiles for the
    channel-attention Gram arrive pre-transposed via DMA (ptd input) and
    the Gram runs in the prologue as the PE warm-up burst; gamma scaling
    is host-folded; wpCB bias vectors run on GPSIMD; the final
    U*(1/denom)+oc combine uses a PE broadcast + 64-partition reciprocal
    (never a 1-partition DVE op) with the add on GPSIMD.
"""

from contextlib import ExitStack

import ml_dtypes
import numpy as np

import concourse.bass as bass
import concourse.mybir as mybir
import concourse.tile as tile
from concourse import bacc
from concourse.bass import ds, ts
from concourse.bass_utils import run_bass_kernel_spmd
from concourse.masks import make_identity
from concourse.tile import add_dep_helper

F32 = mybir.dt.float32
BF16 = mybir.dt.bfloat16
I16 = mybir.dt.int16
AF = mybir.ActivationFunctionType
ALU = mybir.AluOpType
AX = mybir.AxisListType

B, C, D, H, W = 2, 64, 20, 20, 20
N = D * H * W            # 8000
MT = 128                 # key (m) tile size
NRT = 63                 # real m tiles (63*128 = 8064 >= 8000)
NPAD = 8192              # padded key range in pab
HALF = NPAD // 2         # 4096 (m-tile pair split)
NPAIR = 32               # pair iterations (A=i, B=32+i)
NQ = 2000                # queries per core
CH = 500                 # query chunk width (4 chunks)
NCH = 4
KCH = 512                # kp projection chunk
LAVT = 4                 # vt pair lookahead
NCORES = 8
SCH_C = 184.6650390625   # 128/ln(2): bf16 Schraudolph scale
SCH_B = 16256.0          # 127*128


def build_danet(ctx, tc, io):
    nc = tc.nc
    xbb, xq, xqb2, ptd = io["xbb"], io["xq"], io["xqb2"], io["ptd"]
    mpT, wvx, gc, eye2, out_d = (io["mpT"], io["wvx"], io["gc"],
                                 io["eye2"], io["out"])

    persist = ctx.enter_context(tc.tile_pool(name="persist", bufs=1))
    fs_pool = ctx.enter_context(tc.tile_pool(name="fs", bufs=6))
    up = ctx.enter_context(tc.tile_pool(name="ps_u", bufs=1, space="PSUM"))
    fp = ctx.enter_context(tc.tile_pool(name="ps_f", bufs=1, space="PSUM"))

    pab = persist.tile([65, NPAD], BF16)      # bf16 proj + ones row (host)
    paq = persist.tile([64, NQ], F32)         # query block fp32 (outc2)
    paqb2 = persist.tile([128, NQ], BF16)     # query block bf16, duplicated
    kp2 = persist.tile([128, HALF], BF16)     # M@p packed halves
    vt = persist.tile([128, NRT, 66], BF16)   # [gamma_p*vT | 1 | w.p]
    pt = persist.tile([128, NRT, 64], BF16)   # projT tiles (DMA, channel)
    wpcb = persist.tile([128, NRT], F32)      # w.p*C + B (Schraudolph bias)
    mpT_s = persist.tile([64, 64], BF16)
    wvx_s = persist.tile([65, 66], BF16)
    gc_s = persist.tile([64, 1], F32)
    eye2_s = persist.tile([64, 64], F32)
    id64 = persist.tile([64, 64], F32)
    ones_s = persist.tile([1, 64], F32)
    ec_acc = persist.tile([64, 64], F32)
    ee = persist.tile([64, 64], F32)
    ac2 = persist.tile([64, 64], F32)
    mx = persist.tile([64, 1], F32)
    sc = persist.tile([64, 1], F32)
    rc = persist.tile([64, 1], F32)
    rcg = persist.tile([64, 1], F32)
    oc_sb = persist.tile([64, NQ], F32)       # gamma_c*out_c + 2x
    d4 = persist.tile([1, NQ], F32)           # softmax denominators
    rcp = persist.tile([64, 2 * 512], F32)    # 1/denom bcast (ping-pong)
    out_sb = persist.tile([64, NQ], F32)

    # ---- input DMAs (ordered so bootstrap consumers land first) ----
    nc.sync.dma_start(out=mpT_s, in_=mpT)
    nc.sync.dma_start(out=wvx_s, in_=wvx)
    nc.sync.dma_start(out=gc_s, in_=gc)
    nc.sync.dma_start(out=eye2_s, in_=eye2)
    nc.sync.dma_start(out=paqb2, in_=xqb2)
    xw = NPAD // 8
    nc.sync.dma_start(out=pab[:, ts(0, xw)], in_=xbb[:, ts(0, xw)])
    nc.sync.dma_start(out=pab[:, ts(4, xw)], in_=xbb[:, ts(4, xw)])
    nc.sync.dma_start(out=pt, in_=ptd)
    for i in (1, 5, 2, 6, 3, 7):
        nc.sync.dma_start(out=pab[:, ts(i, xw)], in_=xbb[:, ts(i, xw)])
    nc.sync.dma_start(out=paq, in_=xq)
    make_identity(nc, id64)
    nc.vector.memset(ones_s, 1.0)

    tag_n = [0]

    def tagf():
        # rotating transient PSUM tag among the F slots 0..2 (3 is the
        # prologue Gram's; flash F uses all four after the Gram retires)
        tag_n[0] = (tag_n[0] + 1) % 3
        return f"f{tag_n[0]}"

    # ---- PE warm-up burst (while pab/ptd DMAs land) ----
    warm = fp.tile([128, 512], F32, name="warm", tag=tagf())
    for r in range(6):
        nc.tensor.matmul(warm, paqb2[:, 0:128], paqb2[:, 0:512],
                         start=True, stop=True, skip_group_check=True)

    def emit_kp(c, eng):
        """kp2 chunk c (0..15): cols c%8*512 of half c//8."""
        half = c // 8
        sl = slice(half * 64, half * 64 + 64)
        kp_ps = fp.tile([128, KCH], F32, name=f"kp{c}", tag=tagf())
        nc.tensor.matmul(kp_ps[sl, :], mpT_s,
                         pab[0:64, ds(half * HALF + (c % 8) * KCH, KCH)],
                         start=True, stop=True,
                         tile_position=(0, half * 64))
        if eng == "act":
            nc.scalar.copy(out=kp2[sl, ts(c % 8, KCH)], in_=kp_ps[sl, :])
        else:
            nc.vector.tensor_copy(out=kp2[sl, ts(c % 8, KCH)],
                                  in_=kp_ps[sl, :])

    def emit_vt_pair(p):
        """wvx projection for tiles (p, 32+p): one PSUM bank, one copy."""
        tb = 32 + p
        has_b = tb <= NRT - 1
        nt = 2 if has_b else 1
        vt_ps = fp.tile([128, KCH], F32, name=f"vt{p}", tag=tagf())
        nc.tensor.matmul(vt_ps[:, 0:66], pab[:, ts(p, MT)], wvx_s,
                         start=True, stop=True)
        if has_b:
            nc.tensor.matmul(vt_ps[:, 66:132], pab[:, ts(tb, MT)], wvx_s,
                             start=True, stop=True)
        # strided copy into vt rows p and 32+p in one DVE instruction
        nc.vector.tensor_copy(out=vt[:, p:p + 1 + (32 if has_b else 0):32, :],
                              in_=vt_ps[:, 0:nt * 66])
        # Schraudolph per-key bias on GPSIMD (SBUF-only engine)
        nc.gpsimd.tensor_scalar(
            out=wpcb[:, p:p + 1 + (32 if has_b else 0):32],
            in0=vt[:, p:p + 1 + (32 if has_b else 0):32, 65],
            scalar1=SCH_C, scalar2=SCH_B, op0=ALU.mult, op1=ALU.add)

    # ---- prologue: kp + channel-attention Gram (dense PE work) ----
    emit_kp(0, "act")
    emit_kp(8, "dve")
    g_ps = fp.tile([128, 512], F32, name="gram", tag="f3")
    for t in range(NRT):
        nc.tensor.matmul(g_ps[0:64, 0:64], pt[:, t, :], pt[:, t, :],
                         start=(t == 0), stop=(t == NRT - 1))
    nc.vector.tensor_copy(out=ec_acc, in_=g_ps[0:64, 0:64])
    for c in (1, 9, 2, 10, 3, 11, 4, 12, 5, 13, 6, 14, 7, 15):
        emit_kp(c, "act" if c < 8 else "dve")
    for p in range(LAVT):
        emit_vt_pair(p)

    # ---- main flash loop: software pipeline over 128 sub-iters ----
    # sub-iter j = (pair i, chunk c): F leads, exp lags 1, U lags 2.
    u_ps = [up.tile([65, 512], F32, name=f"u{c}", tag=f"u{c}")
            for c in range(NCH)]
    NSUB = NPAIR * NCH
    fsb = [None] * NSUB   # (fsb_a_ap, fsb_b_ap) pending U consumption
    last_exp = [None]

    def emit_F(j):
        i, c = divmod(j, NCH)
        has_b = 32 + i <= NRT - 1
        sa, sb = (2 * j) % 4, (2 * j + 1) % 4
        fa = fp.tile([128, 512], F32, name="fa", tag=f"f{sa}")
        nc.tensor.matmul(fa[:, 0:CH], kp2[0:64, ts(i, MT)],
                         paqb2[0:64, ds(c * CH, CH)],
                         start=True, stop=True, tile_position=(0, 0))
        fb = None
        if has_b:
            fb = fp.tile([128, 512], F32, name="fb", tag=f"f{sb}")
            nc.tensor.matmul(fb[:, 0:CH], kp2[64:128, ts(i, MT)],
                             paqb2[64:128, ds(c * CH, CH)],
                             start=True, stop=True, tile_position=(64, 0))
        return fa, fb

    fps = [None] * NSUB

    def emit_exp(j):
        i, c = divmod(j, NCH)
        fa, fb = fps[j]
        outs = []
        for t, f_ps, dve in ((i, fa, c == 2),
                             (32 + i, fb, c in (1, 3))):
            if f_ps is None:
                outs.append(None)
                continue
            if dve:
                fe = fs_pool.tile([128, 512], I16, name="fsb", tag="fsb")
                e = nc.vector.tensor_scalar(
                    out=fe[:, 0:CH], in0=f_ps[:, 0:CH],
                    scalar1=SCH_C, scalar2=wpcb[:, t:t + 1],
                    op0=ALU.mult, op1=ALU.add)
                outs.append(fe[:, 0:CH].bitcast(BF16))
            else:
                fe = fs_pool.tile([128, 512], BF16, name="fsb", tag="fsb")
                e = nc.scalar.activation(out=fe[:, 0:CH], in_=f_ps[:, 0:CH],
                                         func=AF.Exp, bias=vt[:, t, 65:66])
                last_exp[0] = e
                outs.append(fe[:, 0:CH])
        fsb[j] = outs

    def emit_U(j):
        i, c = divmod(j, NCH)
        ea, eb = fsb[j]
        nc.tensor.matmul(u_ps[c][:, 0:CH], vt[:, i, 0:65], ea,
                         start=(i == 0), stop=(i == NPAIR - 1))
        if eb is not None:
            nc.tensor.matmul(u_ps[c][:, 0:CH], vt[:, 32 + i, 0:65], eb,
                             start=False, stop=False)
        fsb[j] = None

    for step in range(NSUB + 2):
        jf, jx, ju = step, step - 1, step - 2
        if jf < NSUB:
            fps[jf] = emit_F(jf)
        if 0 <= jx < NSUB:
            emit_exp(jx)
        if 0 <= ju < NSUB:
            emit_U(ju)
        if jf < NSUB:
            i, c = divmod(jf, NCH)
            if c == 1 and i + LAVT <= NPAIR - 1:
                emit_vt_pair(i + LAVT)

    # ---- epilogue: channel attention softmax -> ac2 -> outc2 ----
    nc.vector.tensor_reduce(out=mx, in_=ec_acc, axis=AX.X, op=ALU.max,
                            negate=True)
    ee_inst = nc.scalar.activation(out=ee, in_=ec_acc, func=AF.Exp, bias=mx)
    if last_exp[0] is not None:
        add_dep_helper(ee_inst.ins, last_exp[0].ins, sync=False,
                       reason="channel softmax after flash exps")
    nc.vector.tensor_reduce(out=sc, in_=ee, axis=AX.X, op=ALU.add)
    nc.vector.reciprocal(out=rc, in_=sc)
    nc.vector.tensor_mul(out=rcg, in0=rc, in1=gc_s)
    nc.vector.tensor_scalar_mul(out=ee, in0=ee, scalar1=rcg)
    at_ps = fp.tile([64, 64], F32, name="at_ps", tag=tagf())
    nc.tensor.transpose(at_ps, ee, id64)
    nc.vector.tensor_add(out=ac2, in0=at_ps, in1=eye2_s)
    for c in range(NCH):  # outc2 = gamma_c*out_c + 2x (fp32: exact 2x)
        oc_ps = fp.tile([64, CH], F32, name=f"oc{c}", tag=tagf())
        nc.tensor.matmul(oc_ps, ac2, paq[:, ts(c, CH)],
                         start=True, stop=True)
        nc.scalar.copy(out=oc_sb[:, ts(c, CH)], in_=oc_ps)

    # ---- per-chunk combine: out = U[0:64]/U[64] + oc ----
    for c in range(NCH):
        csl = ds(c * CH, CH)
        nc.scalar.copy(out=d4[:, csl], in_=u_ps[c][64:65, 0:CH])
        bc_ps = fp.tile([64, 512], F32, name=f"bc{c}", tag=tagf())
        nc.tensor.matmul(bc_ps[:, 0:CH], ones_s, d4[:, csl],
                         start=True, stop=True)
        rsl = ds((c % 2) * 512, CH)
        nc.vector.reciprocal(out=rcp[:, rsl], in_=bc_ps[:, 0:CH])
        nc.vector.tensor_mul(out=out_sb[:, csl], in0=u_ps[c][0:64, 0:CH],
                             in1=rcp[:, rsl])
        nc.gpsimd.tensor_tensor(out=out_sb[:, csl], in0=out_sb[:, csl],
                                in1=oc_sb[:, csl], op=ALU.add)
        nc.sync.dma_start(out=out_d[:, csl], in_=out_sb[:, csl])


def _mk_io(nc):
    io = {}
    io["xbb"] = nc.dram_tensor("xbb", [65, NPAD], BF16,
                               kind="ExternalInput").ap()
    io["xq"] = nc.dram_tensor("xq", [64, NQ], F32, kind="ExternalInput").ap()
    io["xqb2"] = nc.dram_tensor("xqb2", [128, NQ], BF16,
                                kind="ExternalInput").ap()
    io["ptd"] = nc.dram_tensor("ptd", [128, NRT * 64], BF16,
                               kind="ExternalInput").ap()
    io["mpT"] = nc.dram_tensor("mpT", [64, 64], BF16,
                               kind="ExternalInput").ap()
    io["wvx"] = nc.dram_tensor("wvx", [65, 66], BF16,
                               kind="ExternalInput").ap()
    io["gc"] = nc.dram_tensor("gc", [64, 1], F32, kind="ExternalInput").ap()
    io["eye2"] = nc.dram_tensor("eye2", [64, 64], F32,
                                kind="ExternalInput").ap()
    io["out"] = nc.dram_tensor("out", [64, NQ], F32,
                               kind="ExternalOutput").ap()
    return io


_CACHE = {}


def build_program():
    if "nc" not in _CACHE:
        nc = bacc.Bacc("TRN2", target_bir_lowering=False, debug=False,
                       num_devices=NCORES)
        io = _mk_io(nc)
        with tile.TileContext(nc) as tc, ExitStack() as ctx:
            build_danet(ctx, tc, io)
        nc.compile()
        _CACHE["nc"] = nc
    return _CACHE["nc"]


def make_in_maps(x, Wq, bq, Wk, bk, Wv, bv, gamma_c, gamma_p):
    f = np.float32
    bf = ml_dtypes.bfloat16
    proj = np.asarray(x, f).reshape(B, C, N)
    Wq, bq, Wk, bk = (np.asarray(a, f) for a in (Wq, bq, Wk, bk))
    Wv, bv = np.asarray(Wv, f), np.asarray(bv, f)
    gamma_c = float(np.asarray(gamma_c).reshape(-1)[0])
    gamma_p = float(np.asarray(gamma_p).reshape(-1)[0])

    mpT = (Wq.T @ Wk).T.astype(bf)       # lhsT for kp = M @ p
    w = (Wk.T @ bq).astype(f)            # per-key bias inside softmax
    wvx = np.zeros((65, 66), f)
    wvx[0:64, 0:64] = gamma_p * Wv.T     # gamma_p folded into the weights
    wvx[64, 0:64] = gamma_p * bv
    wvx[64, 64] = 1.0                    # ones column (0 for padded keys)
    wvx[0:64, 65] = w
    wvx = wvx.astype(bf)
    gc = np.full((64, 1), gamma_c, f)
    eye2 = (2.0 * np.eye(64)).astype(f)

    in_maps = []
    for core in range(NCORES):
        b, qb = divmod(core, 4)
        xbuf = np.zeros((65, NPAD), f)
        xbuf[0:64, 0:N] = proj[b]
        xbuf[64, 0:N] = 1.0              # zero beyond N: pads self-cancel
        pp = np.zeros((64, NRT * MT), f)
        pp[:, 0:N] = proj[b]
        ptd = np.ascontiguousarray(
            pp.reshape(64, NRT, MT).transpose(2, 1, 0).reshape(MT, NRT * 64))
        xqf = np.ascontiguousarray(proj[b][:, qb * NQ:(qb + 1) * NQ])
        xqb2 = np.broadcast_to(xqf.astype(bf), (2, 64, NQ)).reshape(128, NQ)
        in_maps.append({"xbb": xbuf.astype(bf), "xq": xqf,
                        "xqb2": np.ascontiguousarray(xqb2),
                        "ptd": ptd.astype(bf), "mpT": mpT,
                        "wvx": wvx, "gc": gc, "eye2": eye2})
    return in_maps


def run_on_cores(in_maps, **kw):
    nc = build_program()
    return run_bass_kernel_spmd(nc, in_maps, core_ids=list(range(NCORES)),
                                **kw)


def kernel(**inputs):
    x = np.asarray(inputs["x"])
    in_maps = make_in_maps(
        inputs["x"], inputs["Wq"], inputs["bq"], inputs["Wk"], inputs["bk"],
        inputs["Wv"], inputs["bv"], inputs["gamma_c"], inputs["gamma_p"])
    res = run_on_cores(in_maps)
    out = np.zeros((B, C, N), np.float32)
    for core in range(NCORES):
        b, qb = divmod(core, 4)
        out[b][:, qb * NQ:(qb + 1) * NQ] = res.results[core]["out"]
    return out.reshape(x.shape).astype(x.dtype, copy=False)


# revision 7
# speedup vs baseline: 1.0867x; 1.0867x over previous
"""DANet3D dual-attention kernel for Trainium2 (8 NeuronCores, Bass/Tile).

Sharding: x -> proj p [2, 64, 8000]; 8 cores = 2 batches x 4 query-blocks
of 2000 positions.  Each core receives the full batch projection (keys /
values / channel attention) plus its own query block and computes its
[64, 2000] slice of the output.

Position attention (per batch), with M = Wq^T Wk, w = Wk^T bq:
  softmax_m( p_n^T M p_m + w.p_m )  ->  flash loop in E^T layout
  F = exp(kp_m . p_n + w.p_m),  kp = M p
  U[65, q] += vt[m, 0:65]^T F[m, q],  vt = [gamma_p*(Wv p + bv) | ones]
  (gamma_p is folded into Wv/bv on the host; the ones column comes from
  the x-buffer's ones row, which is zeroed for padded keys so padding
  self-cancels in both numerator and denominator).

Engine budget (per core, warm): PE ~190k cycles of F/U matmuls is the
long pole; the exp of the 8064x2000 score matrix (16.1M elements) can
only run on ACT (1.2G col/s) and DVE (0.96G col/s) because GPSIMD has no
PSUM port.  The kernel therefore:
  * pipelines F (PSUM slots f0..f3, 2 per sub-iter, reuse distance 2)
    ahead of exp, with U two sub-iters behind, so the PE instruction
    stream has no dependency stalls (HAM stays at K=8/8, 2.4 GHz);
  * splits the 8 exp tiles per key-pair 5:3 between ACT (native Exp with
    the per-key bias in the ACT bias slot) and DVE (Schraudolph exp:
    int16(x*184.665 + (w.p*184.665+16256)) bit-cast to bf16);
  * moves everything else off the critical engines: p^T tiles for the
    channel-attention Gram arrive pre-transposed via DMA (ptd input) and
    the Gram runs in the prologue as the PE warm-up burst; gamma scaling
    is host-folded; wpCB bias vectors run on GPSIMD; the final
    U*(1/denom)+oc combine uses a PE broadcast + 64-partition reciprocal
    (never a 1-partition DVE op) with the add on GPSIMD.
"""

from contextlib import ExitStack

import ml_dtypes
import numpy as np

import concourse.bass as bass
import concourse.mybir as mybir
import concourse.tile as tile
from concourse import bacc
from concourse.bass import ds, ts
from concourse.bass_utils import run_bass_kernel_spmd
from concourse.masks import make_identity
from concourse.tile import add_dep_helper

F32 = mybir.dt.float32
BF16 = mybir.dt.bfloat16
I16 = mybir.dt.int16
AF = mybir.ActivationFunctionType
ALU = mybir.AluOpType
AX = mybir.AxisListType

B, C, D, H, W = 2, 64, 20, 20, 20
N = D * H * W            # 8000
MT = 128                 # key (m) tile size
NRT = 63                 # real m tiles (63*128 = 8064 >= 8000)
NPAD = 8192              # padded key range in pab
HALF = NPAD // 2         # 4096 (m-tile pair split)
NPAIR = 32               # pair iterations (A=i, B=32+i)
NQ = 2000                # queries per core
CH = 500                 # query chunk width (4 chunks)
NCH = 4
KCH = 512                # kp projection chunk
LAVT = 4                 # vt pair lookahead
NCORES = 8
SCH_C = 184.6650390625   # 128/ln(2): bf16 Schraudolph scale
SCH_B = 16256.0          # 127*128


def build_danet(ctx, tc, io):
    nc = tc.nc
    xbb, xq, xqb2, ptd = io["xbb"], io["xq"], io["xqb2"], io["ptd"]
    mpT, wvx, gc, eye2, out_d = (io["mpT"], io["wvx"], io["gc"],
                                 io["eye2"], io["out"])

    persist = ctx.enter_context(tc.tile_pool(name="persist", bufs=1))
    fs_pool = ctx.enter_context(tc.tile_pool(name="fs", bufs=6))
    up = ctx.enter_context(tc.tile_pool(name="ps_u", bufs=1, space="PSUM"))
    fp = ctx.enter_context(tc.tile_pool(name="ps_f", bufs=1, space="PSUM"))

    pab = persist.tile([65, NPAD], BF16)      # bf16 proj + ones row (host)
    paq = persist.tile([64, NQ], F32)         # query block fp32 (outc2)
    paqb2 = persist.tile([128, NQ], BF16)     # query block bf16, duplicated
    kp2 = persist.tile([128, HALF], BF16)     # M@p packed halves
    vt = persist.tile([128, NRT, 66], BF16)   # [gamma_p*vT | 1 | w.p]
    pt = persist.tile([128, NRT, 64], BF16)   # projT tiles (DMA, channel)
    wpcb = persist.tile([128, NRT], F32)      # w.p*C + B (Schraudolph bias)
    mpT_s = persist.tile([64, 64], BF16)
    wvx_s = persist.tile([65, 66], BF16)
    gc_s = persist.tile([64, 1], F32)
    eye2_s = persist.tile([64, 64], F32)
    id64 = persist.tile([64, 64], F32)
    ones_s = persist.tile([1, 64], F32)
    ec_acc = persist.tile([64, 64], F32)
    ee = persist.tile([64, 64], F32)
    ac2 = persist.tile([64, 64], F32)
    mx = persist.tile([64, 1], F32)
    sc = persist.tile([64, 1], F32)
    rc = persist.tile([64, 1], F32)
    rcg = persist.tile([64, 1], F32)
    oc_sb = persist.tile([64, NQ], F32)       # gamma_c*out_c + 2x
    d4 = persist.tile([1, NQ], F32)           # softmax denominators
    rcp = persist.tile([64, 2 * 512], F32)    # 1/denom bcast (ping-pong)
    out_sb = persist.tile([64, NQ], F32)

    # ---- input DMAs (ordered so bootstrap consumers land first) ----
    nc.sync.dma_start(out=mpT_s, in_=mpT)
    nc.sync.dma_start(out=wvx_s, in_=wvx)
    nc.sync.dma_start(out=gc_s, in_=gc)
    nc.sync.dma_start(out=eye2_s, in_=eye2)
    nc.sync.dma_start(out=paqb2, in_=xqb2)
    xw = NPAD // 8
    nc.sync.dma_start(out=pab[:, ts(0, xw)], in_=xbb[:, ts(0, xw)])
    nc.sync.dma_start(out=pab[:, ts(4, xw)], in_=xbb[:, ts(4, xw)])
    nc.sync.dma_start(out=pt, in_=ptd)
    for i in (1, 5, 2, 6, 3, 7):
        nc.sync.dma_start(out=pab[:, ts(i, xw)], in_=xbb[:, ts(i, xw)])
    nc.sync.dma_start(out=paq, in_=xq)
    make_identity(nc, id64)
    nc.vector.memset(ones_s, 1.0)

    tag_n = [0]

    def tagf():
        # rotating transient PSUM tag among the F slots 0..2 (3 is the
        # prologue Gram's; flash F uses all four after the Gram retires)
        tag_n[0] = (tag_n[0] + 1) % 3
        return f"f{tag_n[0]}"

    def emit_kp(c, eng):
        """kp2 chunk c (0..15): cols c%8*512 of half c//8."""
        half = c // 8
        sl = slice(half * 64, half * 64 + 64)
        kp_ps = fp.tile([128, KCH], F32, name=f"kp{c}", tag=tagf())
        nc.tensor.matmul(kp_ps[sl, :], mpT_s,
                         pab[0:64, ds(half * HALF + (c % 8) * KCH, KCH)],
                         start=True, stop=True,
                         tile_position=(0, half * 64))
        if eng == "act":
            nc.scalar.copy(out=kp2[sl, ts(c % 8, KCH)], in_=kp_ps[sl, :])
        else:
            nc.vector.tensor_copy(out=kp2[sl, ts(c % 8, KCH)],
                                  in_=kp_ps[sl, :])

    def emit_vt_pair(p):
        """wvx projection for tiles (p, 32+p): one PSUM bank, one copy."""
        tb = 32 + p
        has_b = tb <= NRT - 1
        nt = 2 if has_b else 1
        vt_ps = fp.tile([128, KCH], F32, name=f"vt{p}", tag=tagf())
        nc.tensor.matmul(vt_ps[:, 0:66], pab[:, ts(p, MT)], wvx_s,
                         start=True, stop=True)
        if has_b:
            nc.tensor.matmul(vt_ps[:, 66:132], pab[:, ts(tb, MT)], wvx_s,
                             start=True, stop=True)
        # strided copy into vt rows p and 32+p in one DVE instruction
        nc.vector.tensor_copy(out=vt[:, p:p + 1 + (32 if has_b else 0):32, :],
                              in_=vt_ps[:, 0:nt * 66])
        # Schraudolph per-key bias on GPSIMD (SBUF-only engine)
        nc.gpsimd.tensor_scalar(
            out=wpcb[:, p:p + 1 + (32 if has_b else 0):32],
            in0=vt[:, p:p + 1 + (32 if has_b else 0):32, 65],
            scalar1=SCH_C, scalar2=SCH_B, op0=ALU.mult, op1=ALU.add)

    # ---- prologue: kp/vt projections, then the channel-attention Gram
    # as a dense 63-matmul burst right before the flash loop so the PE
    # enters the loop un-throttled (HAM K=8/8). ----
    emit_kp(0, "act")
    emit_kp(8, "dve")
    for p in range(LAVT):
        emit_vt_pair(p)
    for c in (1, 9, 2, 10, 3, 11, 4, 12, 5, 13, 6, 14, 7, 15):
        emit_kp(c, "act" if c < 8 else "dve")
    g_ps = fp.tile([128, 512], F32, name="gram", tag="f3")
    for t in range(NRT):
        nc.tensor.matmul(g_ps[0:64, 0:64], pt[:, t, :], pt[:, t, :],
                         start=(t == 0), stop=(t == NRT - 1))
    nc.vector.tensor_copy(out=ec_acc, in_=g_ps[0:64, 0:64])

    # ---- main flash loop: software pipeline over 128 sub-iters ----
    # sub-iter j = (pair i, chunk c): F leads, exp lags 1, U lags 2.
    u_ps = [up.tile([65, 512], F32, name=f"u{c}", tag=f"u{c}")
            for c in range(NCH)]
    NSUB = NPAIR * NCH
    fsb = [None] * NSUB   # (fsb_a_ap, fsb_b_ap) pending U consumption
    last_exp = [None]

    def emit_F(j):
        i, c = divmod(j, NCH)
        has_b = 32 + i <= NRT - 1
        sa, sb = (2 * j) % 4, (2 * j + 1) % 4
        fa = fp.tile([128, 512], F32, name="fa", tag=f"f{sa}")
        nc.tensor.matmul(fa[:, 0:CH], kp2[0:64, ts(i, MT)],
                         paqb2[0:64, ds(c * CH, CH)],
                         start=True, stop=True, tile_position=(0, 0))
        fb = None
        if has_b:
            fb = fp.tile([128, 512], F32, name="fb", tag=f"f{sb}")
            nc.tensor.matmul(fb[:, 0:CH], kp2[64:128, ts(i, MT)],
                             paqb2[64:128, ds(c * CH, CH)],
                             start=True, stop=True, tile_position=(64, 0))
        return fa, fb

    fps = [None] * NSUB

    def emit_exp(j):
        i, c = divmod(j, NCH)
        fa, fb = fps[j]
        outs = []
        for t, f_ps, dve in ((i, fa, c in (0, 2)),
                             (32 + i, fb, c in (1, 3))):
            if f_ps is None:
                outs.append(None)
                continue
            if dve:
                fe = fs_pool.tile([128, 512], I16, name="fsb", tag="fsb")
                e = nc.vector.tensor_scalar(
                    out=fe[:, 0:CH], in0=f_ps[:, 0:CH],
                    scalar1=SCH_C, scalar2=wpcb[:, t:t + 1],
                    op0=ALU.mult, op1=ALU.add)
                outs.append(fe[:, 0:CH].bitcast(BF16))
            else:
                fe = fs_pool.tile([128, 512], BF16, name="fsb", tag="fsb")
                e = nc.scalar.activation(out=fe[:, 0:CH], in_=f_ps[:, 0:CH],
                                         func=AF.Exp, bias=vt[:, t, 65:66])
                last_exp[0] = e
                outs.append(fe[:, 0:CH])
        fsb[j] = outs

    def emit_U(j):
        i, c = divmod(j, NCH)
        ea, eb = fsb[j]
        nc.tensor.matmul(u_ps[c][:, 0:CH], vt[:, i, 0:65], ea,
                         start=(i == 0), stop=(i == NPAIR - 1))
        if eb is not None:
            nc.tensor.matmul(u_ps[c][:, 0:CH], vt[:, 32 + i, 0:65], eb,
                             start=False, stop=False)
        fsb[j] = None

    for step in range(NSUB + 2):
        jf, jx, ju = step, step - 1, step - 2
        if jf < NSUB:
            fps[jf] = emit_F(jf)
        if 0 <= jx < NSUB:
            emit_exp(jx)
        if 0 <= ju < NSUB:
            emit_U(ju)
        if jf < NSUB:
            i, c = divmod(jf, NCH)
            if c == 1 and i + LAVT <= NPAIR - 1:
                emit_vt_pair(i + LAVT)

    # ---- epilogue: channel attention softmax -> ac2 -> outc2 ----
    nc.vector.tensor_reduce(out=mx, in_=ec_acc, axis=AX.X, op=ALU.max,
                            negate=True)
    ee_inst = nc.scalar.activation(out=ee, in_=ec_acc, func=AF.Exp, bias=mx)
    if last_exp[0] is not None:
        add_dep_helper(ee_inst.ins, last_exp[0].ins, sync=False,
                       reason="channel softmax after flash exps")
    nc.vector.tensor_reduce(out=sc, in_=ee, axis=AX.X, op=ALU.add)
    nc.vector.reciprocal(out=rc, in_=sc)
    nc.vector.tensor_mul(out=rcg, in0=rc, in1=gc_s)
    nc.vector.tensor_scalar_mul(out=ee, in0=ee, scalar1=rcg)
    at_ps = fp.tile([64, 64], F32, name="at_ps", tag=tagf())
    nc.tensor.transpose(at_ps, ee, id64)
    nc.vector.tensor_add(out=ac2, in0=at_ps, in1=eye2_s)
    for c in range(NCH):  # outc2 = gamma_c*out_c + 2x (fp32: exact 2x)
        oc_ps = fp.tile([64, CH], F32, name=f"oc{c}", tag=tagf())
        nc.tensor.matmul(oc_ps, ac2, paq[:, ts(c, CH)],
                         start=True, stop=True)
        nc.scalar.copy(out=oc_sb[:, ts(c, CH)], in_=oc_ps)

    # ---- per-chunk combine: out = U[0:64]/U[64] + oc ----
    for c in range(NCH):
        csl = ds(c * CH, CH)
        nc.scalar.copy(out=d4[:, csl], in_=u_ps[c][64:65, 0:CH])
        bc_ps = fp.tile([64, 512], F32, name=f"bc{c}", tag=tagf())
        nc.tensor.matmul(bc_ps[:, 0:CH], ones_s, d4[:, csl],
                         start=True, stop=True)
        rsl = ds((c % 2) * 512, CH)
        nc.vector.reciprocal_approx_fast(out=rcp[:, rsl], in_=bc_ps[:, 0:CH])
        nc.vector.tensor_mul(out=out_sb[:, csl], in0=u_ps[c][0:64, 0:CH],
                             in1=rcp[:, rsl])
        nc.gpsimd.tensor_tensor(out=out_sb[:, csl], in0=out_sb[:, csl],
                                in1=oc_sb[:, csl], op=ALU.add)
        nc.sync.dma_start(out=out_d[:, csl], in_=out_sb[:, csl])


def _mk_io(nc):
    io = {}
    io["xbb"] = nc.dram_tensor("xbb", [65, NPAD], BF16,
                               kind="ExternalInput").ap()
    io["xq"] = nc.dram_tensor("xq", [64, NQ], F32, kind="ExternalInput").ap()
    io["xqb2"] = nc.dram_tensor("xqb2", [128, NQ], BF16,
                                kind="ExternalInput").ap()
    io["ptd"] = nc.dram_tensor("ptd", [128, NRT * 64], BF16,
                               kind="ExternalInput").ap()
    io["mpT"] = nc.dram_tensor("mpT", [64, 64], BF16,
                               kind="ExternalInput").ap()
    io["wvx"] = nc.dram_tensor("wvx", [65, 66], BF16,
                               kind="ExternalInput").ap()
    io["gc"] = nc.dram_tensor("gc", [64, 1], F32, kind="ExternalInput").ap()
    io["eye2"] = nc.dram_tensor("eye2", [64, 64], F32,
                                kind="ExternalInput").ap()
    io["out"] = nc.dram_tensor("out", [64, NQ], F32,
                               kind="ExternalOutput").ap()
    return io


_CACHE = {}


def build_program():
    if "nc" not in _CACHE:
        nc = bacc.Bacc("TRN2", target_bir_lowering=False, debug=False,
                       num_devices=NCORES)
        io = _mk_io(nc)
        with tile.TileContext(nc) as tc, ExitStack() as ctx:
            build_danet(ctx, tc, io)
        nc.compile()
        _CACHE["nc"] = nc
    return _CACHE["nc"]


def make_in_maps(x, Wq, bq, Wk, bk, Wv, bv, gamma_c, gamma_p):
    f = np.float32
    bf = ml_dtypes.bfloat16
    proj = np.asarray(x, f).reshape(B, C, N)
    Wq, bq, Wk, bk = (np.asarray(a, f) for a in (Wq, bq, Wk, bk))
    Wv, bv = np.asarray(Wv, f), np.asarray(bv, f)
    gamma_c = float(np.asarray(gamma_c).reshape(-1)[0])
    gamma_p = float(np.asarray(gamma_p).reshape(-1)[0])

    mpT = (Wq.T @ Wk).T.astype(bf)       # lhsT for kp = M @ p
    w = (Wk.T @ bq).astype(f)            # per-key bias inside softmax
    wvx = np.zeros((65, 66), f)
    wvx[0:64, 0:64] = gamma_p * Wv.T     # gamma_p folded into the weights
    wvx[64, 0:64] = gamma_p * bv
    wvx[64, 64] = 1.0                    # ones column (0 for padded keys)
    wvx[0:64, 65] = w
    wvx = wvx.astype(bf)
    gc = np.full((64, 1), gamma_c, f)
    eye2 = (2.0 * np.eye(64)).astype(f)

    in_maps = []
    for core in range(NCORES):
        b, qb = divmod(core, 4)
        xbuf = np.zeros((65, NPAD), f)
        xbuf[0:64, 0:N] = proj[b]
        xbuf[64, 0:N] = 1.0              # zero beyond N: pads self-cancel
        pp = np.zeros((64, NRT * MT), f)
        pp[:, 0:N] = proj[b]
        ptd = np.ascontiguousarray(
            pp.reshape(64, NRT, MT).transpose(2, 1, 0).reshape(MT, NRT * 64))
        xqf = np.ascontiguousarray(proj[b][:, qb * NQ:(qb + 1) * NQ])
        xqb2 = np.broadcast_to(xqf.astype(bf), (2, 64, NQ)).reshape(128, NQ)
        in_maps.append({"xbb": xbuf.astype(bf), "xq": xqf,
                        "xqb2": np.ascontiguousarray(xqb2),
                        "ptd": ptd.astype(bf), "mpT": mpT,
                        "wvx": wvx, "gc": gc, "eye2": eye2})
    return in_maps


def run_on_cores(in_maps, **kw):
    nc = build_program()
    return run_bass_kernel_spmd(nc, in_maps, core_ids=list(range(NCORES)),
                                **kw)


def kernel(**inputs):
    x = np.asarray(inputs["x"])
    in_maps = make_in_maps(
        inputs["x"], inputs["Wq"], inputs["bq"], inputs["Wk"], inputs["bk"],
        inputs["Wv"], inputs["bv"], inputs["gamma_c"], inputs["gamma_p"])
    res = run_on_cores(in_maps)
    out = np.zeros((B, C, N), np.float32)
    for core in range(NCORES):
        b, qb = divmod(core, 4)
        out[b][:, qb * NQ:(qb + 1) * NQ] = res.results[core]["out"]
    return out.reshape(x.shape).astype(x.dtype, copy=False)
